# revision 6
# baseline (speedup 1.0000x reference)
"""Trainium2 Bass kernel for nn_AttentiveEncoder (embed -> linear -> full self-attention).

Input-distribution-minimal sharding: the dominant cost of this problem is
shipping bytes to the 8 cores, so the host routes only what each core needs.

  host:  core c receives e_rows = emb_table[ids[1024c : 1024c+1024]]  (4 MB)
         and a 128-row k-shard of W.T (512 KB); emb table itself is never
         replicated (1.05 GB -> 37 MB total upload).
  device, phase A: AllGather the W.T shards -> full W.T; PE-transpose E;
         L = E @ W.T + b  and  L^T = W^T @ E^T + b^T  computed as two
         fp32r matmul series from the same operands (L^T is a matmul, not a
         transpose), both written bf16 and staged to local DRAM per 256-row
         chunk so the first collectives start early.
  exchange: per chunk g (256 rows/core), TWO AllGathers: natural-layout
         L rows -> gath_nat[2048, 1024] and transposed L^T columns ->
         gath_t[8*1024, 256]. No SWDGE transpose gathers, no DMA transposes:
         the transposed copy rides the collective.
  attention: per chunk, kv_nat (natural) and kv_t (transposed, [h, j]) are
         SBUF-resident. Per (q-group 256, key j-tile 128): S^T = K^T.T @ Q^T
         on the tensor engine, P = exp(S/sqrt(H)) bf16 on ACT, then
         out += P.T @ V and den += P.T @ 1 accumulated in PSUM per chunk.
         DVE flushes PSUM into SBUF accumulators; the last chunk's flush
         also normalizes (reciprocal of den) and stores f32.

Q^T is the core's own slice of L^T (SBUF-resident; no DRAM round trip).
"""
import numpy as np
from contextlib import ExitStack

import concourse.bass as bass
import concourse.bacc as bacc
import concourse.tile as tile
from concourse.tile import add_dep_helper
from concourse import mybir
from concourse.bass_utils import run_bass_kernel_spmd

F32 = mybir.dt.float32
F32R = mybir.dt.float32r
BF16 = mybir.dt.bfloat16

N_CORES = 8
VOCAB = 32000
H = 1024             # hidden
SEQ = 8192           # sequence
NQ = SEQ // N_CORES  # query rows per core (1024)
KT = H // 128        # k-tiles over hidden (8)
CHUNK = 256          # rows per core per collective chunk
G = NQ // CHUNK      # chunks (4)
QGS = 256            # query rows per q-group
NQG = NQ // QGS      # q-groups per core (4)
IC = QGS // 128      # i-chunks per q-group (2)
HC = H // 512        # h-chunks (2)
SCALE = 1.0 / np.sqrt(np.float32(H))

_cached = None


def _build(sim_single_core=False):
    nc = bacc.Bacc()

    e_rows = nc.dram_tensor("e_rows", [NQ, H], F32, kind="ExternalInput")
    wt_s = nc.dram_tensor("wt_s", [128, H], F32, kind="ExternalInput")  # W.T k-shard
    bias = nc.dram_tensor("bias", [1, H], F32, kind="ExternalInput")
    ident = nc.dram_tensor("ident", [128, 128], F32, kind="ExternalInput")
    out_d = nc.dram_tensor("out", [NQ, H], F32, kind="ExternalOutput")

    with tile.TileContext(nc) as tc, ExitStack() as ctx:
        pers = ctx.enter_context(tc.tile_pool(name="pers", bufs=1))
        dram = ctx.enter_context(tc.tile_pool(name="dram", bufs=1, space="DRAM"))

        lt_sb = pers.tile([128, KT, NQ], BF16, tag="lt_sb")   # L^T = Q^T [h, i]
        out_acc = pers.tile([128, NQ // 128, H], F32, tag="out_acc")
        den_acc = pers.tile([128, NQ // 128], F32, tag="den_acc")
        ones_bf = pers.tile([128, 1], BF16, tag="ones_bf")
        nc.vector.memset(ones_bf[:], 1.0)

        l_stage = dram.tile([NQ, H], BF16)                    # local L (natural)
        lt_stage = dram.tile([G, H, CHUNK], BF16)             # local L^T chunk-major
        wt_full = dram.tile([H, H], F32, addr_space="Shared", name="wt_full")
        gath_n = [dram.tile([N_CORES * CHUNK, H], BF16, addr_space="Shared",
                            name=f"gnat{g}") for g in range(G)]
        gath_t = [dram.tile([N_CORES * H, CHUNK], BF16, addr_space="Shared",
                            name=f"gt{g}") for g in range(G)]

        # ---------------- phase A ----------------
        if not sim_single_core:
            wt_sstage = dram.tile([128, H], F32, name="wt_sstage")
            nc.sync.dma_start(wt_sstage[:], wt_s[:])
            nc.gpsimd.collective_compute(
                "AllGather", mybir.AluOpType.bypass,
                replica_groups=[list(range(N_CORES))],
                ins=[wt_sstage[:]], outs=[wt_full.opt()],
            )

        with tc.tile_pool(name="pa", bufs=1) as pa, \
             tc.tile_pool(name="pa_tp", bufs=2, space="PSUM") as pa_tp:
            e_nat = pa.tile([128, NQ // 128, H], F32, tag="e_nat")
            nc.sync.dma_start(e_nat[:], e_rows.rearrange("(a p) h -> p a h", p=128))
            id_sb = pa.tile([128, 128], F32, tag="id_sb")
            nc.sync.dma_start(id_sb[:], ident[:])
            b_sb = pa.tile([1, H], F32, tag="b_sb")
            nc.sync.dma_start(b_sb[:], bias[:])
            b_r = pa.tile([1, H], F32R, tag="b_r")
            nc.vector.tensor_copy(b_r[:], b_sb[:])
            one_f = pa.tile([1, 512], F32, tag="one_f")
            nc.vector.memset(one_f[:], 1.0)
            one_r = pa.tile([1, 512], F32R, tag="one_r")
            nc.vector.tensor_copy(one_r[:], one_f[:])

            # E^T via PE transposes (needed as lhsT/rhs for both L matmuls)
            e_t = pa.tile([128, KT, NQ], F32R, tag="e_t")
            for it in range(NQ // 128):
                for kt in range(KT):
                    tp = pa_tp.tile([128, 128], F32, tag="tp")
                    nc.tensor.transpose(tp[:], e_nat[:, it, kt * 128:(kt + 1) * 128],
                                        id_sb[:])
                    nc.vector.tensor_copy(e_t[:, kt, it * 128:(it + 1) * 128], tp[:])

            w_sb = pa.tile([128, KT, H], F32, tag="w_sb")
            if sim_single_core:
                for kt in range(KT):
                    nc.sync.dma_start(w_sb[:, kt, :], wt_s[:])
            else:
                nc.sync.dma_start(w_sb[:], wt_full.rearrange("(kt p) h -> p kt h", p=128))
            w_r = pa.tile([128, KT, H], F32R, tag="w_r")
            nc.vector.tensor_copy(w_r[:], w_sb[:])

            l_bf = pa.tile([128, NQ // 128, H], BF16, tag="l_bf")
            l_stage_r = l_stage.rearrange("(a p) h -> p a h", p=128)
            lt_stage_r = lt_stage.rearrange("g (kt p) c -> g p kt c", p=128)

            with tc.tile_pool(name="pa_ps", bufs=1, space="PSUM") as pa_ps, \
                 tc.tile_pool(name="pa_ps2", bufs=1, space="PSUM") as pa_ps2:
                for g in range(G):
                    # natural L for this chunk's two i-tiles
                    for half in range(2):
                        it = 2 * g + half
                        ps = pa_ps.tile([128, HC, 512], F32, tag="ps")
                        for hc in range(HC):
                            for kt in range(KT):
                                nc.tensor.matmul(
                                    ps[:, hc, :],
                                    e_t[:, kt, it * 128:(it + 1) * 128],
                                    w_r[:, kt, hc * 512:(hc + 1) * 512],
                                    start=(kt == 0), stop=False,
                                )
                            nc.tensor.matmul(
                                ps[:, hc, :], one_r[:, 0:128],
                                b_r[:, hc * 512:(hc + 1) * 512],
                                start=False, stop=True,
                            )
                            nc.scalar.copy(l_bf[:, it, hc * 512:(hc + 1) * 512],
                                           ps[:, hc, :])
                    # transposed L^T for this chunk's 256 columns
                    pst = pa_ps2.tile([128, KT, CHUNK], F32, tag="pst")
                    for ht in range(KT):
                        for kt in range(KT):
                            nc.tensor.matmul(
                                pst[:, ht, :],
                                w_r[:, kt, ht * 128:(ht + 1) * 128],
                                e_t[:, kt, g * CHUNK:(g + 1) * CHUNK],
                                start=(kt == 0), stop=False,
                            )
                        nc.tensor.matmul(
                            pst[:, ht, :], b_r[:, ht * 128:(ht + 1) * 128],
                            one_r[:, 0:CHUNK],
                            start=False, stop=True,
                        )
                    nc.scalar.copy(lt_sb[:, :, g * CHUNK:(g + 1) * CHUNK], pst[:])
                    # stage both layouts (ACT HWDGE queues, separate from SP loads)
                    nc.scalar.dma_start(l_stage_r[:, 2 * g:2 * g + 2, :],
                                        l_bf[:, 2 * g:2 * g + 2, :])
                    nc.scalar.dma_start(lt_stage_r[g],
                                        lt_sb[:, :, g * CHUNK:(g + 1) * CHUNK])
                    if g == 0 and not sim_single_core:
                        nc.gpsimd.collective_compute(
                            "AllGather", mybir.AluOpType.bypass,
                            replica_groups=[list(range(N_CORES))],
                            ins=[l_stage[0:CHUNK, :]], outs=[gath_n[0].opt()],
                        )
                        nc.gpsimd.collective_compute(
                            "AllGather", mybir.AluOpType.bypass,
                            replica_groups=[list(range(N_CORES))],
                            ins=[lt_stage[0]], outs=[gath_t[0].opt()],
                        )

        # ---------------- chunked collectives + attention ----------------
        nblk = 1 if sim_single_core else N_CORES
        with tc.tile_pool(name="kv", bufs=2) as kvp, \
             tc.tile_pool(name="pt", bufs=4) as ptp, \
             tc.tile_pool(name="st_ps", bufs=2, space="PSUM") as st_ps, \
             tc.tile_pool(name="out_ps", bufs=1, space="PSUM") as out_ps, \
             tc.tile_pool(name="den_ps", bufs=1, space="PSUM") as den_ps, \
             tc.tile_pool(name="fin", bufs=2) as fin:
            for g in range(G):
                CJ = nblk * CHUNK  # keys per chunk
                if sim_single_core:
                    kv_nat = kvp.tile([128, CJ // 128, H], BF16, tag="kv_nat",
                                      name=f"kv_nat{g}")
                    ld_nat = nc.sync.dma_start(
                        kv_nat[:],
                        l_stage[g * CHUNK:(g + 1) * CHUNK, :]
                        .rearrange("(a p) h -> p a h", p=128))
                    kv_t = kvp.tile([128, KT, CJ], BF16, tag="kv_t",
                                    name=f"kv_t{g}")
                    nc.sync.dma_start(
                        kv_t[:], lt_stage_r[g])
                else:
                    kv_nat = kvp.tile([128, CJ // 128, H], BF16, tag="kv_nat",
                                      name=f"kv_nat{g}")
                    ld_nat = nc.sync.dma_start(
                        kv_nat[:],
                        gath_n[g][0:CJ, :].rearrange("(a p) h -> p a h", p=128))
                    kv_t = kvp.tile([128, KT, CJ], BF16, tag="kv_t",
                                    name=f"kv_t{g}")
                    ld_t = None
                    for r in range(N_CORES):
                        ld_t = nc.sync.dma_start(
                            kv_t[:, :, r * CHUNK:(r + 1) * CHUNK],
                            gath_t[g][r * H:(r + 1) * H, :]
                            .rearrange("(kt p) c -> p kt c", p=128))
                    if g + 1 < G:
                        agn = nc.gpsimd.collective_compute(
                            "AllGather", mybir.AluOpType.bypass,
                            replica_groups=[list(range(N_CORES))],
                            ins=[l_stage[(g + 1) * CHUNK:(g + 2) * CHUNK, :]],
                            outs=[gath_n[g + 1].opt()],
                        )
                        add_dep_helper(agn.ins, ld_nat.ins, sync=False,
                                       reason="AG after this chunk's loads")
                        agt = nc.gpsimd.collective_compute(
                            "AllGather", mybir.AluOpType.bypass,
                            replica_groups=[list(range(N_CORES))],
                            ins=[lt_stage[g + 1]], outs=[gath_t[g + 1].opt()],
                        )
                        add_dep_helper(agt.ins, ld_t.ins, sync=False,
                                       reason="AG after this chunk's loads")

                for qg in range(NQG):
                    ops = out_ps.tile([128, 2 * HC, 512], F32, tag="ops")
                    dps = [den_ps.tile([128, 1], F32, tag=f"dps{ic}",
                                       name=f"dps{g}_{qg}_{ic}") for ic in range(IC)]
                    NJT = CJ // 128  # j-tiles per chunk
                    for jt in range(NJT):
                        st = st_ps.tile([128, QGS], F32, tag="st")
                        for ht in range(KT):
                            nc.tensor.matmul(
                                st[:],
                                kv_t[:, ht, jt * 128:(jt + 1) * 128],
                                lt_sb[:, ht, qg * QGS:(qg + 1) * QGS],
                                start=(ht == 0), stop=(ht == KT - 1),
                            )
                        p_t = ptp.tile([128, QGS], BF16, tag="p_t")
                        nc.scalar.activation(p_t[:], st[:],
                                             mybir.ActivationFunctionType.Exp,
                                             scale=float(SCALE))
                        first, last = (jt == 0), (jt == NJT - 1)
                        for ic in range(IC):
                            lhs = p_t[:, ic * 128:(ic + 1) * 128]
                            for hc in range(HC):
                                nc.tensor.matmul(
                                    ops[:, ic * HC + hc, :],
                                    lhs, kv_nat[:, jt, hc * 512:(hc + 1) * 512],
                                    start=first, stop=last,
                                )
                            nc.tensor.matmul(
                                dps[ic][:], lhs, ones_bf[:],
                                start=first, stop=last,
                            )
                    # flush psum accumulators into SBUF accumulators;
                    # last chunk: normalize + store immediately
                    out_r = out_d.rearrange("(a p) h -> p a h", p=128)
                    for ic in range(IC):
                        gi = qg * IC + ic
                        acc = out_acc[:, gi, :]
                        pslice = ops[:, ic * HC:(ic + 1) * HC, :]
                        if g == 0:
                            nc.vector.tensor_copy(acc, pslice.opt())
                            nc.vector.tensor_copy(den_acc[:, gi:gi + 1], dps[ic][:])
                        else:
                            nc.vector.tensor_add(acc, acc, pslice.opt())
                            nc.vector.tensor_add(den_acc[:, gi:gi + 1],
                                                 den_acc[:, gi:gi + 1], dps[ic][:])
                        if g == G - 1:
                            recip = pers.tile([128, 1], F32, tag=f"recip{gi}",
                                              name=f"recip{gi}")
                            nc.vector.reciprocal(recip[:], den_acc[:, gi:gi + 1])
                            o = fin.tile([128, H], F32, tag="o")
                            nc.vector.tensor_scalar_mul(o[:], acc, recip[:])
                            nc.sync.dma_start(out_r[:, gi, :], o[:])

    nc.compile()
    return nc


def _get_nc():
    global _cached
    if _cached is None:
        _cached = _build()
    return _cached


last_results = None
_last_in_maps = None


def kernel(input, emb_table, W, b):
    global last_results, _last_in_maps
    nc = _get_nc()

    ids = np.asarray(input).astype(np.int64)
    emb_np = np.asarray(emb_table, dtype=np.float32)
    wt_np = np.ascontiguousarray(np.asarray(W, dtype=np.float32).T)  # [k, h]
    b_np = np.ascontiguousarray(np.asarray(b, dtype=np.float32).reshape(1, H))
    ident_np = np.eye(128, dtype=np.float32)

    in_maps = []
    for c in range(N_CORES):
        rows = emb_np[ids[c * NQ:(c + 1) * NQ]]        # [NQ, H] host gather
        wt_shard = wt_np[c * 128:(c + 1) * 128]         # [128, H] contiguous view
        in_maps.append({
            "e_rows": rows, "wt_s": wt_shard, "bias": b_np, "ident": ident_np,
        })

    _last_in_maps = in_maps
    res = run_bass_kernel_spmd(nc, in_maps, list(range(N_CORES)))
    last_results = res
    return np.concatenate([res.results[c]["out"] for c in range(N_CORES)], axis=0)
